# revision 20
# baseline (speedup 1.0000x reference)
"""Trainium2 Bass kernel for batched tiny-projection attention.

Reference computation (per batch b):
    qp = relu(q @ W1.T + b1)            [Nq, 3]
    kp = relu(k @ W2.T + b2)            [Nf, 3]
    scores = (qp @ kp.T) / sqrt(3)      [Nq, Nf]
    attn = softmax(scores, axis=-1)
    out = attn @ v                      [Nq, C]

Shapes: B=4, Nq=2048, Nf=16384, D=3, C=768, fp32.

Sharding: 8 cores = (4 batches) x (2 halves of Nq). Each core handles
q[b, h*1024:(h+1)*1024], full k[b]/v[b], so softmax is local to a core
(no cross-core reduction needed).

The tiny D=3 projections (0.1% of the FLOPs) are folded into host-side
input prep (the host already computes qp/kp for the exp-shift bound):
  - qsp [128, Nq] fp16: exact hi/lo split of fp32 qp, hi components
    at partitions {0-2}, lo at {32-34}, zeros elsewhere.
  - k16f [128, Nf] fp16: fp16(kp) replicated at partitions {0-2, 32-34};
    one K=128 matmul then contracts k16.(qhi+qlo) = k16.qp with only the
    ~2^-11 k-rounding as error.

Device algorithm (per core), oriented for the tensor engine:
  - scores are computed TRANSPOSED: sT[m, n] = kp[m]. qp[n], because the
    attn @ v matmul needs the contraction dim (m) on partitions.
  - exp(scale*s - shift) runs on the scalar engine straight out of PSUM,
    emitting bf16 tiles (bf16 range avoids underflow for rows whose max
    score is far below the global shift; scores >= 0 since qp,kp >= 0).
  - attn @ v accumulates in PSUM over a group of m-tiles, then is
    flushed (added) into an SBUF fp32 accumulator; v carries an extra
    ones column so the softmax denominator falls out of the same matmul.
  - Dummy matmuls at t=0 warm the PE HAM clock gate (to 2.4 GHz) while
    the head DMAs land; a dummy exp preloads the ACT spline table.
  - The last groups are small-ish (8+8) with the per-chunk normalize +
    output DMA fused in, so the 3 MB store overlaps compute.
"""

import sys

sys.path.insert(0, "/opt/trn_rl_repo")

import numpy as np

import concourse.bass as bass
import concourse.bacc as bacc
import concourse.tile as tile
from concourse import mybir
from concourse.bass_utils import run_bass_kernel_spmd

F32 = mybir.dt.float32
F16 = mybir.dt.float16
BF16 = mybir.dt.bfloat16

B, NQ_FULL, NF, D, C = 4, 2048, 16384, 3, 768
SCALE = 1.0 / np.sqrt(3.0)
NQ = NQ_FULL // 2          # per-core query rows
CA, CB = 512, C + 1 - 512  # c-chunk split of [v | ones] (769 = 512 + 257)


def build_nc(nq=NQ, nf=NF, g=16, num_devices=8):
    """Build the single-core SPMD program. g = m-tiles (of 128) per group."""
    assert nq % 512 == 0 and nf % 128 == 0
    m_tiles = nf // 128
    nchunks = nq // 128
    gm = g * 128            # field rows per group (max)
    caug = C + 1

    nc = bacc.Bacc("TRN2", target_bir_lowering=False, debug=False,
                   num_devices=num_devices)

    qsp = nc.dram_tensor("qsp", [128, nq], F16, kind="ExternalInput")
    k16f = nc.dram_tensor("k16f", [128, nf], F16, kind="ExternalInput")
    vaug = nc.dram_tensor("vaug", [nf, caug], BF16, kind="ExternalInput")
    shift = nc.dram_tensor("shift", [128, 1], F32, kind="ExternalInput")
    out = nc.dram_tensor("out", [nq, C], F32, kind="ExternalOutput")

    with tile.TileContext(nc) as tc, \
         tc.tile_pool(name="const", bufs=1) as const, \
         tc.tile_pool(name="k16p", bufs=3) as k16p, \
         tc.tile_pool(name="vp", bufs=2 * g) as vp, \
         tc.tile_pool(name="expp", bufs=2 * g) as expp, \
         tc.tile_pool(name="outp", bufs=3) as outp, \
         tc.tile_pool(name="recp", bufs=3) as recp, \
         tc.tile_pool(name="sc_ps", bufs=4, space="PSUM") as sc_ps, \
         tc.tile_pool(name="oA_ps", bufs=2, space="PSUM") as oA_ps, \
         tc.tile_pool(name="oB_ps", bufs=2, space="PSUM") as oB_ps:

        # ---- PE warm-up: dummy matmuls fill the HAM activity window
        # while the head DMAs land, so the ramp runs at 2.4 GHz ----
        warm_in = const.tile([128, 256], F16)
        nc.gpsimd.memset(warm_in[:], 0.0)
        warm_ps = sc_ps.tile([128, 512], F32, tag="sp")
        for _ in range(20):
            nc.tensor.matmul(warm_ps[:, 0:256], warm_in[:, 0:128],
                             warm_in[:], start=True, stop=True)
        # preload the scalar engine's activation table set off the
        # critical path (first exp would otherwise eat ~1.3us)
        warm_act = const.tile([128, 1], F32)
        nc.scalar.activation(warm_act[:], warm_in[:, 0:1],
                             mybir.ActivationFunctionType.Exp, scale=1.0)

        # ---- prologue: head DMAs, critical ones on the parallel queue
        qsplit = const.tile([128, nq], F16)
        nc.scalar.dma_start(qsplit[:], qsp[:])
        shift_sb = const.tile([128, 1], F32)
        nc.sync.dma_start(shift_sb[:], shift[:])

        acc = const.tile([128, nchunks, caug], F32)

        def emit_kt(m0_tiles, size, engine=None):
            n = size * 128
            kt = k16p.tile([128, gm], F16)
            c0 = m0_tiles * 128
            (engine or nc.sync).dma_start(kt[:, 0:n], k16f[:, c0:c0 + n])
            return kt

        def emit_v(m0_tiles, size):
            vts = []
            for t in range(size):
                m0 = (m0_tiles + t) * 128
                vt = vp.tile([128, caug], BF16)
                nc.sync.dma_start(vt[:], vaug[m0:m0 + 128, :])
                vts.append(vt)
            return vts

        def emit_scores(k16, ts, h_major=False):
            """scores + exp for m-tiles ts (local idx within group).
            h_major orders the low n-columns of every tile first, so the
            first attn chunk's dependencies complete earliest."""
            es = []
            for t in ts:
                et = expp.tile([128, nq], BF16)
                es.append(et)
            ts = list(ts)
            order = [(h, j) for h in range(nq // 512) for j in range(len(ts))]
            if not h_major:
                order = [(h, j) for j in range(len(ts)) for h in range(nq // 512)]
            for h, j in order:
                t = ts[j]
                sp = sc_ps.tile([128, 512], F32, tag="sp")
                nc.tensor.matmul(sp[:], k16[:, t * 128:(t + 1) * 128],
                                 qsplit[:, h * 512:(h + 1) * 512],
                                 start=True, stop=True)
                nc.scalar.activation(es[j][:, h * 512:(h + 1) * 512], sp[:],
                                     mybir.ActivationFunctionType.Exp,
                                     bias=shift_sb[:], scale=float(SCALE))
            return es

        def emit_attn_chunk(first_group, ci, es, vts, den_first=False):
            n = len(es)
            pA = oA_ps.tile([128, CA], F32)
            pB = oB_ps.tile([128, CB], F32)
            for i in range(n):
                e = es[i][:, ci * 128:(ci + 1) * 128]
                nc.tensor.matmul(pA[:], e, vts[i][:, 0:CA],
                                 start=(i == 0), stop=(i == n - 1))
                nc.tensor.matmul(pB[:], e, vts[i][:, CA:caug],
                                 start=(i == 0), stop=(i == n - 1))
            if first_group:
                nc.vector.tensor_copy(acc[:, ci, 0:CA], pA[:])
                nc.vector.tensor_copy(acc[:, ci, CA:caug], pB[:])
            elif den_first:
                nc.vector.tensor_add(acc[:, ci, CA:caug], acc[:, ci, CA:caug],
                                     pB[:])
                nc.vector.tensor_add(acc[:, ci, 0:CA], acc[:, ci, 0:CA], pA[:])
            else:
                nc.vector.tensor_add(acc[:, ci, 0:CA], acc[:, ci, 0:CA], pA[:])
                nc.vector.tensor_add(acc[:, ci, CA:caug], acc[:, ci, CA:caug],
                                     pB[:])

        def emit_finale(ci):
            rec = recp.tile([128, 1], F32)
            nc.vector.reciprocal(rec[:], acc[:, ci, C:caug])
            ot = outp.tile([128, C], F32)
            for c0 in (0, C // 2):
                nc.vector.tensor_scalar_mul(ot[:, c0:c0 + C // 2],
                                            acc[:, ci, c0:c0 + C // 2],
                                            rec[:])
                nc.sync.dma_start(out[ci * 128:(ci + 1) * 128, c0:c0 + C // 2],
                                  ot[:, c0:c0 + C // 2])

        # ---- software-pipelined main loop ----
        # small groups first (attn starts waiting on only a few exp
        # tiles); the last groups leave DVE room for the fused finale.
        if m_tiles == 128 and g == 16:
            sizes = [2, 4, 10] + [16] * 6 + [8, 8]
        else:
            ngroups = m_tiles // g
            assert g * ngroups == m_tiles
            sizes = [g] * ngroups
        starts = [sum(sizes[:i]) for i in range(len(sizes))]
        n_g = len(sizes)

        ks = {0: emit_kt(starts[0], sizes[0], engine=nc.scalar)}
        v_cur = emit_v(starts[0], sizes[0])
        if n_g > 1:
            ks[1] = emit_kt(starts[1], sizes[1])
        e_cur = emit_scores(ks[0], range(sizes[0]), h_major=True)

        for gi in range(n_g):
            last = gi + 1 >= n_g
            if gi + 2 < n_g:
                ks[gi + 2] = emit_kt(starts[gi + 2], sizes[gi + 2])
            if not last:
                v_nxt = emit_v(starts[gi + 1], sizes[gi + 1])
                e_nxt = []
            # distribute next group's score matmuls across this group's
            # attn chunks to keep PE dense and ACT fed early
            for ci in range(nchunks):
                emit_attn_chunk(gi == 0, ci, e_cur, v_cur,
                                den_first=last)
                if last:
                    emit_finale(ci)
                else:
                    nnx = sizes[gi + 1]
                    per = (nnx + nchunks - 1) // nchunks
                    ts = range(ci * per, min((ci + 1) * per, nnx))
                    e_nxt.extend(emit_scores(ks[gi + 1], ts))
            if not last:
                v_cur, e_cur = v_nxt, e_nxt

    nc.finalize()
    return nc


def _host_prep(q, k, v, W1, b1, W2, b2):
    """Build per-core input maps (tiny projections + layout/dtype prep)."""
    import ml_dtypes

    in_maps = []
    per_batch = {}
    qp_full = {}
    for b in range(B):
        qp = np.maximum(q[b].astype(np.float32) @ W1.T.astype(np.float32)
                        + b1.astype(np.float32), 0.0)
        kp = np.maximum(k[b].astype(np.float32) @ W2.T.astype(np.float32)
                        + b2.astype(np.float32), 0.0)
        bound = SCALE * float(qp.max(axis=0) @ kp.max(axis=0))
        va = np.ones((NF, C + 1), np.float32)
        va[:, :C] = v[b]
        kp16 = kp.T.astype(np.float16)          # [3, Nf]
        k16f = np.zeros((128, NF), np.float16)
        k16f[0:3] = kp16
        k16f[32:35] = kp16
        per_batch[b] = {
            "k16f": k16f,
            "vaug": va.astype(ml_dtypes.bfloat16),
            "shift": np.full((128, 1), -bound, np.float32),
        }
        qp_full[b] = qp
    for core in range(8):
        b, h = core // 2, core % 2
        qp = qp_full[b][h * NQ:(h + 1) * NQ].T   # [3, NQ] fp32
        hi = qp.astype(np.float16)
        lo = (qp - hi.astype(np.float32)).astype(np.float16)
        qsp = np.zeros((128, NQ), np.float16)
        qsp[0:3] = hi
        qsp[32:35] = lo
        in_maps.append({"qsp": qsp, **per_batch[b]})
    return in_maps


_NC_CACHE = {}


def kernel(q, k, v, W1, b1, W2, b2, _trace=False):
    q, k, v = np.asarray(q), np.asarray(k), np.asarray(v)
    W1, b1 = np.asarray(W1), np.asarray(b1)
    W2, b2 = np.asarray(W2), np.asarray(b2)

    if "nc" not in _NC_CACHE:
        _NC_CACHE["nc"] = build_nc()
    nc = _NC_CACHE["nc"]

    in_maps = _host_prep(q, k, v, W1, b1, W2, b2)
    res = run_bass_kernel_spmd(nc, in_maps, list(range(8)), trace=_trace)

    out = np.empty((B, NQ_FULL, C), np.float32)
    for core in range(8):
        b, h = core // 2, core % 2
        out[b, h * NQ:(h + 1) * NQ, :] = res.results[core]["out"]
    if _trace:
        return out, res
    return out


# revision 21
# speedup vs baseline: 1.0014x; 1.0014x over previous
"""Trainium2 Bass kernel for batched tiny-projection attention.

Reference computation (per batch b):
    qp = relu(q @ W1.T + b1)            [Nq, 3]
    kp = relu(k @ W2.T + b2)            [Nf, 3]
    scores = (qp @ kp.T) / sqrt(3)      [Nq, Nf]
    attn = softmax(scores, axis=-1)
    out = attn @ v                      [Nq, C]

Shapes: B=4, Nq=2048, Nf=16384, D=3, C=768, fp32.

Sharding: 8 cores = (4 batches) x (2 halves of Nq). Each core handles
q[b, h*1024:(h+1)*1024], full k[b]/v[b], so softmax is local to a core
(no cross-core reduction needed).

The tiny D=3 projections (0.1% of the FLOPs) are folded into host-side
input prep (the host already computes qp/kp for the exp-shift bound):
  - qsp [128, Nq] fp16: exact hi/lo split of fp32 qp, hi components
    at partitions {0-2}, lo at {32-34}, zeros elsewhere.
  - k16f [128, Nf] fp16: fp16(kp) replicated at partitions {0-2, 32-34};
    one K=128 matmul then contracts k16.(qhi+qlo) = k16.qp with only the
    ~2^-11 k-rounding as error.

Device algorithm (per core), oriented for the tensor engine:
  - scores are computed TRANSPOSED: sT[m, n] = kp[m]. qp[n], because the
    attn @ v matmul needs the contraction dim (m) on partitions.
  - exp(scale*s - shift) runs on the scalar engine straight out of PSUM,
    emitting bf16 tiles (bf16 range avoids underflow for rows whose max
    score is far below the global shift; scores >= 0 since qp,kp >= 0).
  - attn @ v accumulates in PSUM over a group of m-tiles, then is
    flushed (added) into an SBUF fp32 accumulator; v carries an extra
    ones column so the softmax denominator falls out of the same matmul.
  - Dummy matmuls at t=0 warm the PE HAM clock gate (to 2.4 GHz) while
    the head DMAs land; a dummy exp preloads the ACT spline table.
  - The last groups are small-ish (8+8) with the per-chunk normalize +
    output DMA fused in, so the 3 MB store overlaps compute.
"""

import sys

sys.path.insert(0, "/opt/trn_rl_repo")

import numpy as np

import concourse.bass as bass
import concourse.bacc as bacc
import concourse.tile as tile
from concourse import mybir
from concourse.bass_utils import run_bass_kernel_spmd

F32 = mybir.dt.float32
F16 = mybir.dt.float16
BF16 = mybir.dt.bfloat16

B, NQ_FULL, NF, D, C = 4, 2048, 16384, 3, 768
SCALE = 1.0 / np.sqrt(3.0)
NQ = NQ_FULL // 2          # per-core query rows
CA, CB = 512, C + 1 - 512  # c-chunk split of [v | ones] (769 = 512 + 257)


def build_nc(nq=NQ, nf=NF, g=16, num_devices=8):
    """Build the single-core SPMD program. g = m-tiles (of 128) per group."""
    assert nq % 512 == 0 and nf % 128 == 0
    m_tiles = nf // 128
    nchunks = nq // 128
    gm = g * 128            # field rows per group (max)
    caug = C + 1

    nc = bacc.Bacc("TRN2", target_bir_lowering=False, debug=False,
                   num_devices=num_devices)

    qsp = nc.dram_tensor("qsp", [128, nq], F16, kind="ExternalInput")
    k16f = nc.dram_tensor("k16f", [128, nf], F16, kind="ExternalInput")
    vaug = nc.dram_tensor("vaug", [nf, caug], BF16, kind="ExternalInput")
    shift = nc.dram_tensor("shift", [128, 1], F32, kind="ExternalInput")
    out = nc.dram_tensor("out", [nq, C], F32, kind="ExternalOutput")

    with tile.TileContext(nc) as tc, \
         tc.tile_pool(name="const", bufs=1) as const, \
         tc.tile_pool(name="k16p", bufs=3) as k16p, \
         tc.tile_pool(name="vp", bufs=2 * g) as vp, \
         tc.tile_pool(name="expp", bufs=2 * g) as expp, \
         tc.tile_pool(name="outp", bufs=3) as outp, \
         tc.tile_pool(name="recp", bufs=3) as recp, \
         tc.tile_pool(name="sc_ps", bufs=4, space="PSUM") as sc_ps, \
         tc.tile_pool(name="oA_ps", bufs=2, space="PSUM") as oA_ps, \
         tc.tile_pool(name="oB_ps", bufs=2, space="PSUM") as oB_ps:

        # ---- PE warm-up: dummy matmuls fill the HAM activity window
        # while the head DMAs land, so the ramp runs at 2.4 GHz ----
        warm_in = const.tile([128, 256], F16)
        nc.gpsimd.memset(warm_in[:], 0.0)
        warm_ps = sc_ps.tile([128, 512], F32, tag="sp")
        for _ in range(20):
            nc.tensor.matmul(warm_ps[:, 0:256], warm_in[:, 0:128],
                             warm_in[:], start=True, stop=True)
        # preload the scalar engine's activation table set off the
        # critical path (first exp would otherwise eat ~1.3us)
        warm_act = const.tile([128, 1], F32)
        nc.scalar.activation(warm_act[:], warm_in[:, 0:1],
                             mybir.ActivationFunctionType.Exp, scale=1.0)

        # ---- prologue: head DMAs, critical ones on the parallel queue
        qsplit = const.tile([128, nq], F16)
        nc.scalar.dma_start(qsplit[:], qsp[:])
        shift_sb = const.tile([128, 1], F32)
        nc.sync.dma_start(shift_sb[:], shift[:])

        acc = const.tile([128, nchunks, caug], F32)

        def emit_kt(m0_tiles, size, engine=None):
            n = size * 128
            kt = k16p.tile([128, gm], F16)
            c0 = m0_tiles * 128
            (engine or nc.sync).dma_start(kt[:, 0:n], k16f[:, c0:c0 + n])
            return kt

        def emit_v(m0_tiles, size):
            vts = []
            for t in range(size):
                m0 = (m0_tiles + t) * 128
                vt = vp.tile([128, caug], BF16)
                nc.sync.dma_start(vt[:], vaug[m0:m0 + 128, :])
                vts.append(vt)
            return vts

        def emit_scores(k16, ts, h_major=False):
            """scores + exp for m-tiles ts (local idx within group).
            h_major orders the low n-columns of every tile first, so the
            first attn chunk's dependencies complete earliest."""
            es = []
            for t in ts:
                et = expp.tile([128, nq], BF16)
                es.append(et)
            ts = list(ts)
            order = [(h, j) for h in range(nq // 512) for j in range(len(ts))]
            if not h_major:
                order = [(h, j) for j in range(len(ts)) for h in range(nq // 512)]
            for h, j in order:
                t = ts[j]
                sp = sc_ps.tile([128, 512], F32, tag="sp")
                nc.tensor.matmul(sp[:], k16[:, t * 128:(t + 1) * 128],
                                 qsplit[:, h * 512:(h + 1) * 512],
                                 start=True, stop=True)
                nc.scalar.activation(es[j][:, h * 512:(h + 1) * 512], sp[:],
                                     mybir.ActivationFunctionType.Exp,
                                     bias=shift_sb[:], scale=float(SCALE))
            return es

        def emit_attn_chunk(first_group, ci, es, vts):
            n = len(es)
            pA = oA_ps.tile([128, CA], F32)
            pB = oB_ps.tile([128, CB], F32)
            for i in range(n):
                e = es[i][:, ci * 128:(ci + 1) * 128]
                nc.tensor.matmul(pA[:], e, vts[i][:, 0:CA],
                                 start=(i == 0), stop=(i == n - 1))
                nc.tensor.matmul(pB[:], e, vts[i][:, CA:caug],
                                 start=(i == 0), stop=(i == n - 1))
            if first_group:
                nc.vector.tensor_copy(acc[:, ci, 0:CA], pA[:])
                nc.vector.tensor_copy(acc[:, ci, CA:caug], pB[:])
            else:
                nc.vector.tensor_add(acc[:, ci, 0:CA], acc[:, ci, 0:CA], pA[:])
                nc.vector.tensor_add(acc[:, ci, CA:caug], acc[:, ci, CA:caug],
                                     pB[:])

        def emit_finale(ci):
            rec = recp.tile([128, 1], F32)
            nc.vector.reciprocal(rec[:], acc[:, ci, C:caug])
            ot = outp.tile([128, C], F32)
            for c0 in (0, C // 2):
                nc.vector.tensor_scalar_mul(ot[:, c0:c0 + C // 2],
                                            acc[:, ci, c0:c0 + C // 2],
                                            rec[:])
                nc.sync.dma_start(out[ci * 128:(ci + 1) * 128, c0:c0 + C // 2],
                                  ot[:, c0:c0 + C // 2])

        # ---- software-pipelined main loop ----
        # small groups first (attn starts waiting on only a few exp
        # tiles); the last groups leave DVE room for the fused finale.
        if m_tiles == 128 and g == 16:
            sizes = [4, 4, 8] + [16] * 6 + [8, 8]
        else:
            ngroups = m_tiles // g
            assert g * ngroups == m_tiles
            sizes = [g] * ngroups
        starts = [sum(sizes[:i]) for i in range(len(sizes))]
        n_g = len(sizes)

        ks = {0: emit_kt(starts[0], sizes[0], engine=nc.scalar)}
        v_cur = emit_v(starts[0], sizes[0])
        if n_g > 1:
            ks[1] = emit_kt(starts[1], sizes[1])
        e_cur = emit_scores(ks[0], range(sizes[0]), h_major=True)

        for gi in range(n_g):
            last = gi + 1 >= n_g
            if gi + 2 < n_g:
                ks[gi + 2] = emit_kt(starts[gi + 2], sizes[gi + 2])
            if not last:
                v_nxt = emit_v(starts[gi + 1], sizes[gi + 1])
                e_nxt = []
            # distribute next group's score matmuls across this group's
            # attn chunks to keep PE dense and ACT fed early
            for ci in range(nchunks):
                emit_attn_chunk(gi == 0, ci, e_cur, v_cur)
                if last:
                    emit_finale(ci)
                else:
                    nnx = sizes[gi + 1]
                    per = (nnx + nchunks - 1) // nchunks
                    ts = range(ci * per, min((ci + 1) * per, nnx))
                    e_nxt.extend(emit_scores(ks[gi + 1], ts))
            if not last:
                v_cur, e_cur = v_nxt, e_nxt

    nc.finalize()
    return nc


def _host_prep(q, k, v, W1, b1, W2, b2):
    """Build per-core input maps (tiny projections + layout/dtype prep)."""
    import ml_dtypes

    in_maps = []
    per_batch = {}
    qp_full = {}
    for b in range(B):
        qp = np.maximum(q[b].astype(np.float32) @ W1.T.astype(np.float32)
                        + b1.astype(np.float32), 0.0)
        kp = np.maximum(k[b].astype(np.float32) @ W2.T.astype(np.float32)
                        + b2.astype(np.float32), 0.0)
        bound = SCALE * float(qp.max(axis=0) @ kp.max(axis=0))
        va = np.ones((NF, C + 1), np.float32)
        va[:, :C] = v[b]
        kp16 = kp.T.astype(np.float16)          # [3, Nf]
        k16f = np.zeros((128, NF), np.float16)
        k16f[0:3] = kp16
        k16f[32:35] = kp16
        per_batch[b] = {
            "k16f": k16f,
            "vaug": va.astype(ml_dtypes.bfloat16),
            "shift": np.full((128, 1), -bound, np.float32),
        }
        qp_full[b] = qp
    for core in range(8):
        b, h = core // 2, core % 2
        qp = qp_full[b][h * NQ:(h + 1) * NQ].T   # [3, NQ] fp32
        hi = qp.astype(np.float16)
        lo = (qp - hi.astype(np.float32)).astype(np.float16)
        qsp = np.zeros((128, NQ), np.float16)
        qsp[0:3] = hi
        qsp[32:35] = lo
        in_maps.append({"qsp": qsp, **per_batch[b]})
    return in_maps


_NC_CACHE = {}


def kernel(q, k, v, W1, b1, W2, b2, _trace=False):
    q, k, v = np.asarray(q), np.asarray(k), np.asarray(v)
    W1, b1 = np.asarray(W1), np.asarray(b1)
    W2, b2 = np.asarray(W2), np.asarray(b2)

    if "nc" not in _NC_CACHE:
        _NC_CACHE["nc"] = build_nc()
    nc = _NC_CACHE["nc"]

    in_maps = _host_prep(q, k, v, W1, b1, W2, b2)
    res = run_bass_kernel_spmd(nc, in_maps, list(range(8)), trace=_trace)

    out = np.empty((B, NQ_FULL, C), np.float32)
    for core in range(8):
        b, h = core // 2, core % 2
        out[b, h * NQ:(h + 1) * NQ, :] = res.results[core]["out"]
    if _trace:
        return out, res
    return out
